# revision 1
# baseline (speedup 1.0000x reference)
"""LocalAttention Bass kernel for Trainium2 (8 NeuronCores).

Problem: B=4 H=8 T=8192 D=64, window=128, look_backward=1, causal.
Sharding: pure (B*H) data parallelism — 32 heads / 8 cores = 4 heads each,
processed as 2 head-pairs so q/k DMAs use all 128 SBUF partitions.

Device algorithm (per head, per 128-token window w):
  S^T[k, q] = K_w' @ Q_w^T      (keys on partitions, so the softmax
                                 reduction over keys can ride the PV matmul)
  P = exp(S^T * D^-0.5) * causal01
  [O^T; r] = [V | 1]^T @ P      (ones column baked into V gives row-sums)
Host divides O^T by r and transposes back.

PSUM "pairblock" layout: one [128, 256] matmul per window computes
[T1(w) | T0(w+1)] = K_w vs [Q_w | Q_{w+1}] — 4 matmuls per 4-window group,
all 256-aligned (no PSUM bank crossing). Window w's prev-block T0(w) is
read from the previous group's P tile; the very first window reads a
constant zero tile (its prev window is fully masked padding).

Host-side shard prep (inside kernel(), numpy):
  qTp [2, 128, 8320]  — head-pair Q^T (head A rows 0-63, head B rows
                        64-127), one zero window appended (lookahead pad)
  kT  [2, 128, 8192]  — head-pair K^T
  vp  [4, 128, 4225]  — per head: 65 V slots x [V(64) | 1], slot 0 zeroed
  mask01 [128, 128]   — within-window causal 0/1 (k <= q)
Output:
  outT [4, 65, 8192]  — rows 0..63 unnormalized O^T, row 64 row-sums r
"""

import numpy as np

B, H, T, D = 4, 8, 8192, 64
W = 128                     # window size
WIN = T // W                # 64 windows per head
NCORES = 8
BH = B * H                  # 32
BH_PER_CORE = BH // NCORES  # 4
NPAIR = BH_PER_CORE // 2    # 2 head pairs per core
CHUNK_W = 32                # windows per load chunk
NCHUNK = WIN // CHUNK_W     # 2
G = 4                       # windows per softmax group (PSUM tile = [128, 1024])
SCALE = float(D) ** -0.5

MASK_ON_GPSIMD = True

_nc_cache = {}
last_perf = None


def _build_nc(skip=()):
    import concourse.tile as tile
    from concourse import bacc
    from concourse import mybir
    from contextlib import ExitStack

    f32 = mybir.dt.float32
    Exp = mybir.ActivationFunctionType.Exp
    mult = mybir.AluOpType.mult

    nc = bacc.Bacc()
    qTp = nc.dram_tensor("qTp", [NPAIR, W, (WIN + 1) * W], f32,
                         kind="ExternalInput")
    kT = nc.dram_tensor("kT", [NPAIR, W, T], f32, kind="ExternalInput")
    vp = nc.dram_tensor("vp", [BH_PER_CORE, W, (WIN + 1) * (D + 1)], f32,
                        kind="ExternalInput")
    mask = nc.dram_tensor("mask01", [W, W], f32, kind="ExternalInput")
    outT = nc.dram_tensor("outT", [BH_PER_CORE, D + 1, T], f32,
                          kind="ExternalOutput")

    with tile.TileContext(nc) as tc, ExitStack() as ctx:
        cpool = ctx.enter_context(tc.tile_pool(name="cpool", bufs=1))
        qpool = ctx.enter_context(tc.tile_pool(name="qpool", bufs=2))
        kpool = ctx.enter_context(tc.tile_pool(name="kpool", bufs=2))
        vpool = ctx.enter_context(tc.tile_pool(name="vpool", bufs=4))
        opool = ctx.enter_context(tc.tile_pool(name="opool", bufs=3))
        ppool = ctx.enter_context(tc.tile_pool(name="ppool", bufs=4))
        spsum = ctx.enter_context(tc.tile_pool(name="spsum", bufs=2, space="PSUM"))
        opsum = ctx.enter_context(tc.tile_pool(name="opsum", bufs=3, space="PSUM"))

        mtile = cpool.tile([W, W], f32)
        nc.sync.dma_start(mtile[:], mask[:])
        z128 = cpool.tile([W, W], f32)       # P for the all-masked pad window
        nc.vector.memset(z128[:], 0.0)

        mm = nc.tensor.matmul
        gidx = 0
        for p in range(NPAIR):
            # per-head P tile of the previous group (for cross-group T0 reads)
            pt_prev = [None, None]
            for c in range(NCHUNK):
                c0 = c * CHUNK_W * W
                qc = qpool.tile([W, (CHUNK_W + 1) * W], f32, tag="qc")
                kc = kpool.tile([W, CHUNK_W * W], f32, tag="kc")
                if "loads" not in skip:
                    nc.sync.dma_start(qc[:], qTp[p, :, c0:c0 + (CHUNK_W + 1) * W])
                    nc.sync.dma_start(kc[:], kT[p, :, c0:c0 + CHUNK_W * W])
                vcs = []
                ocs = []
                for h in range(2):
                    vc = vpool.tile([W, (CHUNK_W + 1) * (D + 1)], f32, tag="vc")
                    if "loads" not in skip:
                        v0 = c * CHUNK_W * (D + 1)
                        nc.sync.dma_start(
                            vc[:],
                            vp[2 * p + h, :, v0:v0 + (CHUNK_W + 1) * (D + 1)])
                    vcs.append(vc)
                    oc = opool.tile([D + 1, CHUNK_W * W], f32, tag="oc")
                    ocs.append(oc)

                for g in range(CHUNK_W // G):
                    w0 = g * G
                    for h in range(2):
                        hb = h * 64  # partition base of this head in qc/kc
                        vc = vcs[h]
                        # pairblock matmuls: [T1(w) | T0(w+1)] per window
                        sp = spsum.tile([W, G * 2 * W], f32, tag="sp")
                        if "smm" not in skip:
                            for i in range(G):
                                wl = w0 + i
                                mm(sp[:, i * 256:(i + 1) * 256],
                                   kc[hb:hb + 64, wl * W:(wl + 1) * W],
                                   qc[hb:hb + 64, wl * W:(wl + 2) * W],
                                   start=True, stop=True)

                        pt = ppool.tile([W, G * 2 * W], f32, tag="pt")
                        if "exp" not in skip:
                            nc.scalar.activation(pt[:], sp[:], Exp, scale=SCALE)

                        # causal mask on T1 blocks (cols 0,256,512,768)
                        pt3 = pt[:].rearrange("p (g x) -> p g x", x=2 * W)
                        t1 = pt3[:, :, 0:W]
                        mb = mtile[:, None, :].to_broadcast([W, G, W])
                        if "mask" not in skip:
                            if MASK_ON_GPSIMD and gidx % 2 == 0:
                                nc.gpsimd.tensor_tensor(t1, t1, mb, mult)
                            else:
                                nc.vector.tensor_tensor(t1, t1, mb, mult)

                        # PV + row-sums
                        op = opsum.tile([D + 1, G * W], f32, tag="op")
                        if "pv" not in skip:
                            for i in range(G):
                                wl = w0 + i
                                if i > 0:
                                    t0src = pt[:, i * 256 - W:i * 256]
                                elif pt_prev[h] is not None:
                                    t0src = pt_prev[h][:, G * 256 - W:G * 256]
                                else:
                                    t0src = z128[:]
                                mm(op[:, i * W:(i + 1) * W],
                                   vc[:, wl * (D + 1):(wl + 1) * (D + 1)],
                                   t0src, start=True, stop=False)
                                mm(op[:, i * W:(i + 1) * W],
                                   vc[:, (wl + 1) * (D + 1):(wl + 2) * (D + 1)],
                                   pt[:, i * 256:i * 256 + W],
                                   start=False, stop=True)

                        if "ocopy" not in skip:
                            nc.vector.tensor_copy(
                                ocs[h][:, w0 * W:(w0 + G) * W], op[:])
                        pt_prev[h] = pt
                        gidx += 1

                if "store" not in skip:
                    for h in range(2):
                        nc.sync.dma_start(
                            outT[2 * p + h, :, c0:c0 + CHUNK_W * W], ocs[h][:])
    nc.finalize()
    return nc


def _prep_core_inputs(q2, k2, v2, core):
    s0 = core * BH_PER_CORE
    qTp = np.zeros((NPAIR, W, (WIN + 1) * W), np.float32)
    kTp = np.zeros((NPAIR, W, T), np.float32)
    for p in range(NPAIR):
        for h in range(2):
            bh = s0 + 2 * p + h
            qTp[p, h * 64:(h + 1) * 64, :T] = q2[bh].T
            kTp[p, h * 64:(h + 1) * 64, :] = k2[bh].T
    vr = v2[s0:s0 + BH_PER_CORE].reshape(
        BH_PER_CORE, WIN, W, D).transpose(0, 2, 1, 3)
    vp = np.zeros((BH_PER_CORE, W, WIN + 1, D + 1), np.float32)
    vp[:, :, 1:, :D] = vr
    vp[:, :, :, D] = 1.0
    vp = np.ascontiguousarray(vp.reshape(BH_PER_CORE, W, (WIN + 1) * (D + 1)))
    mask01 = (np.arange(W)[:, None] <= np.arange(W)[None, :]).astype(np.float32)
    return {"qTp": qTp, "kT": kTp, "vp": vp, "mask01": mask01}


def kernel(q, k, v, _trace=False):
    global last_perf
    from concourse.bass_utils import run_bass_kernel_spmd

    q = np.ascontiguousarray(np.asarray(q), dtype=np.float32)
    k = np.ascontiguousarray(np.asarray(k), dtype=np.float32)
    v = np.ascontiguousarray(np.asarray(v), dtype=np.float32)
    q2 = q.reshape(BH, T, D)
    k2 = k.reshape(BH, T, D)
    v2 = v.reshape(BH, T, D)

    if "nc" not in _nc_cache:
        _nc_cache["nc"] = _build_nc()
    nc = _nc_cache["nc"]

    in_maps = [_prep_core_inputs(q2, k2, v2, core) for core in range(NCORES)]
    res = run_bass_kernel_spmd(
        nc, in_maps, core_ids=list(range(NCORES)), trace=_trace)
    last_perf = res

    outs = []
    for core in range(NCORES):
        ot = res.results[core]["outT"]                 # [4, 65, T]
        o = ot[:, :D, :] / ot[:, D:D + 1, :]           # normalize
        outs.append(o.transpose(0, 2, 1))              # [4, T, 64]
    full = np.concatenate(outs, axis=0)                # [32, T, 64]
    return full.reshape(B, H, T, D)



# revision 9
# speedup vs baseline: 2.1130x; 2.1130x over previous
"""LocalAttention Bass kernel for Trainium2 (8 NeuronCores).

Problem: B=4 H=8 T=8192 D=64, window=128, look_backward=1, causal.
Sharding: pure (B*H) data parallelism — 32 heads / 8 cores = 4 heads each,
processed as 2 head-pairs so q/k DMAs use all 128 SBUF partitions.

Device algorithm (per head, per 128-token window w):
  S^T[k, q] = K_w' @ Q_w^T      (keys on partitions, so the softmax
                                 reduction over keys can ride the PV matmul)
  P = exp(S^T * D^-0.5) * causal01
  [O^T; r] = [V | 1]^T @ P      (ones column baked into V gives row-sums)
Host divides O^T by r and transposes back.

v2 performance notes (vs the fp32 baseline at 305 us):
  - all matmul operands bf16 (fp32 matmul = 4 cyc/col vs 1 for bf16)
  - S matmuls of the two heads in a pair are interleaved so the PE row-
    tiling (head A rows 0-63, head B rows 64-127) runs them concurrently
  - exp writes P directly as bf16; causal mask multiply in bf16
  - PSUM output copy downcasts to bf16 (halves HBM writes); masks run
    1/3 on vector + 2/3 on gpsimd so vector mostly does output copies

PSUM "pairblock" layout: one [128, 256] matmul per window computes
[T1(w) | T0(w+1)] = K_w vs [Q_w | Q_{w+1}] — 4 matmuls per 4-window group,
all 256-aligned (no PSUM bank crossing). Window w's prev-block T0(w) is
read from the previous group's P tile; the very first window reads a
constant zero tile (its prev window is fully masked padding).

Host-side shard prep (inside kernel(), numpy, all bf16):
  qTp [2, 128, 8320]  — head-pair Q^T (head A rows 0-63, head B rows
                        64-127), one zero window appended (lookahead pad)
  kT  [2, 128, 8192]  — head-pair K^T
  vp  [4, 128, 4225]  — per head: 65 V slots x [V(64) | 1], slot 0 zeroed
  mask01 [128, 128]   — within-window causal 0/1 (k <= q)
Output:
  outT [4, 65, 8192] bf16 — rows 0..63 unnormalized O^T, row 64 row-sums
"""

import numpy as np

B, H, T, D = 4, 8, 8192, 64
W = 128                     # window size
WIN = T // W                # 64 windows per head
NCORES = 8
BH = B * H                  # 32
BH_PER_CORE = BH // NCORES  # 4
NPAIR = BH_PER_CORE // 2    # 2 head pairs per core
CHUNK_W = 32                # windows per load chunk
NCHUNK = WIN // CHUNK_W     # 2
G = 4                       # windows per softmax group (PSUM tile = [128, 1024])
SCALE = float(D) ** -0.5

MASK_VECTOR_RATIO = 1       # of every 3 mask multiplies, 1 on vector, 2 gpsimd

_nc_cache = {}
last_perf = None


def _build_nc(skip=()):
    import concourse.tile as tile
    from concourse import bacc
    from concourse import mybir
    from contextlib import ExitStack

    f32 = mybir.dt.float32
    bf16 = mybir.dt.bfloat16
    Exp = mybir.ActivationFunctionType.Exp
    mult = mybir.AluOpType.mult

    nc = bacc.Bacc()
    qTp = nc.dram_tensor("qTp", [NPAIR, W, (WIN + 1) * W], bf16,
                         kind="ExternalInput")
    kT = nc.dram_tensor("kT", [NPAIR, W, T], bf16, kind="ExternalInput")
    vp = nc.dram_tensor("vp", [BH_PER_CORE, W, (WIN + 1) * (D + 1)], bf16,
                        kind="ExternalInput")
    mask = nc.dram_tensor("mask01", [W, W], bf16, kind="ExternalInput")
    outT = nc.dram_tensor("outT", [BH_PER_CORE, D + 1, T], bf16,
                          kind="ExternalOutput")

    with tile.TileContext(nc) as tc, ExitStack() as ctx:
        cpool = ctx.enter_context(tc.tile_pool(name="cpool", bufs=1))
        qpool = ctx.enter_context(tc.tile_pool(name="qpool", bufs=2))
        kpool = ctx.enter_context(tc.tile_pool(name="kpool", bufs=2))
        vpool = ctx.enter_context(tc.tile_pool(name="vpool", bufs=4))
        opool = ctx.enter_context(tc.tile_pool(name="opool", bufs=3))
        ppool = ctx.enter_context(tc.tile_pool(name="ppool", bufs=4))
        spsum = ctx.enter_context(tc.tile_pool(name="spsum", bufs=1, space="PSUM"))
        opsum = ctx.enter_context(tc.tile_pool(name="opsum", bufs=3, space="PSUM"))

        mtile = cpool.tile([W, W], bf16)
        nc.sync.dma_start(mtile[:], mask[:])
        z128 = cpool.tile([W, W], bf16)      # P for the all-masked pad window
        nc.vector.memset(z128[:], 0.0)

        mm = nc.tensor.matmul
        gidx = 0
        for p in range(NPAIR):
            # per-head P tile of the previous group (for cross-group T0 reads)
            pt_prev = [None, None]
            for c in range(NCHUNK):
                c0 = c * CHUNK_W * W
                qc = qpool.tile([W, (CHUNK_W + 1) * W], bf16, tag="qc")
                kc = kpool.tile([W, CHUNK_W * W], bf16, tag="kc")
                if "loads" not in skip:
                    nc.sync.dma_start(qc[:], qTp[p, :, c0:c0 + (CHUNK_W + 1) * W])
                    nc.sync.dma_start(kc[:], kT[p, :, c0:c0 + CHUNK_W * W])
                vcs = []
                ocs = []
                for h in range(2):
                    vc = vpool.tile([W, (CHUNK_W + 1) * (D + 1)], bf16, tag="vc")
                    if "loads" not in skip:
                        v0 = c * CHUNK_W * (D + 1)
                        nc.sync.dma_start(
                            vc[:],
                            vp[2 * p + h, :, v0:v0 + (CHUNK_W + 1) * (D + 1)])
                    vcs.append(vc)
                    oc = opool.tile([D + 1, CHUNK_W * W], bf16, tag="oc")
                    ocs.append(oc)

                for g in range(CHUNK_W // G):
                    w0 = g * G
                    # pairblock matmuls for BOTH heads interleaved: head A
                    # contracts on PE rows 0-63, head B on rows 64-127, so
                    # adjacent instructions row-tile and run concurrently.
                    sp0 = spsum.tile([W, G * 2 * W], f32, tag="sp0")
                    sp1 = spsum.tile([W, G * 2 * W], f32, tag="sp1")
                    sps = [sp0, sp1]
                    if "smm" not in skip:
                        for i in range(G):
                            wl = w0 + i
                            for h in range(2):
                                hb = h * 64
                                mm(sps[h][:, i * 256:(i + 1) * 256],
                                   kc[hb:hb + 64, wl * W:(wl + 1) * W],
                                   qc[hb:hb + 64, wl * W:(wl + 2) * W],
                                   start=True, stop=True)

                    for h in range(2):
                        hb = h * 64
                        vc = vcs[h]
                        pt = ppool.tile([W, G * 2 * W], bf16, tag=f"pt{h}")
                        if "exp" not in skip:
                            nc.scalar.activation(pt[:], sps[h][:], Exp,
                                                 scale=SCALE)

                        # causal mask on T1 blocks (cols 0,256,512,768)
                        pt3 = pt[:].rearrange("p (g x) -> p g x", x=2 * W)
                        t1 = pt3[:, :, 0:W]
                        mb = mtile[:, None, :].to_broadcast([W, G, W])
                        if "mask" not in skip:
                            if gidx % 3 < MASK_VECTOR_RATIO:
                                nc.vector.tensor_tensor(t1, t1, mb, mult)
                            else:
                                nc.gpsimd.tensor_tensor(t1, t1, mb, mult)

                        # PV + row-sums
                        op = opsum.tile([D + 1, G * W], f32, tag="op")
                        if "pv" not in skip:
                            for i in range(G):
                                wl = w0 + i
                                if i > 0:
                                    t0src = pt[:, i * 256 - W:i * 256]
                                elif pt_prev[h] is not None:
                                    t0src = pt_prev[h][:, G * 256 - W:G * 256]
                                else:
                                    t0src = z128[:]
                                mm(op[:, i * W:(i + 1) * W],
                                   vc[:, wl * (D + 1):(wl + 1) * (D + 1)],
                                   t0src, start=True, stop=False)
                                mm(op[:, i * W:(i + 1) * W],
                                   vc[:, (wl + 1) * (D + 1):(wl + 2) * (D + 1)],
                                   pt[:, i * 256:i * 256 + W],
                                   start=False, stop=True)

                        if "ocopy" not in skip:
                            nc.vector.tensor_copy(
                                ocs[h][:, w0 * W:(w0 + G) * W], op[:])
                        pt_prev[h] = pt
                        gidx += 1

                if "store" not in skip:
                    for h in range(2):
                        nc.sync.dma_start(
                            outT[2 * p + h, :, c0:c0 + CHUNK_W * W], ocs[h][:])
    nc.finalize()
    return nc


def _prep_core_inputs(q2, k2, v2, core):
    import ml_dtypes
    bf16 = ml_dtypes.bfloat16
    s0 = core * BH_PER_CORE
    qTp = np.zeros((NPAIR, W, (WIN + 1) * W), bf16)
    kTp = np.zeros((NPAIR, W, T), bf16)
    for p in range(NPAIR):
        for h in range(2):
            bh = s0 + 2 * p + h
            qTp[p, h * 64:(h + 1) * 64, :T] = q2[bh].T.astype(bf16)
            kTp[p, h * 64:(h + 1) * 64, :] = k2[bh].T.astype(bf16)
    vr = v2[s0:s0 + BH_PER_CORE].reshape(
        BH_PER_CORE, WIN, W, D).transpose(0, 2, 1, 3)
    vp = np.zeros((BH_PER_CORE, W, WIN + 1, D + 1), bf16)
    vp[:, :, 1:, :D] = vr.astype(bf16)
    vp[:, :, :, D] = 1.0
    vp = np.ascontiguousarray(vp.reshape(BH_PER_CORE, W, (WIN + 1) * (D + 1)))
    mask01 = (np.arange(W)[:, None] <= np.arange(W)[None, :]).astype(bf16)
    return {"qTp": qTp, "kT": kTp, "vp": vp, "mask01": mask01}


def kernel(q, k, v, _trace=False):
    global last_perf
    from concourse.bass_utils import run_bass_kernel_spmd

    q = np.ascontiguousarray(np.asarray(q), dtype=np.float32)
    k = np.ascontiguousarray(np.asarray(k), dtype=np.float32)
    v = np.ascontiguousarray(np.asarray(v), dtype=np.float32)
    q2 = q.reshape(BH, T, D)
    k2 = k.reshape(BH, T, D)
    v2 = v.reshape(BH, T, D)

    if "nc" not in _nc_cache:
        _nc_cache["nc"] = _build_nc()
    nc = _nc_cache["nc"]

    in_maps = [_prep_core_inputs(q2, k2, v2, core) for core in range(NCORES)]
    res = run_bass_kernel_spmd(
        nc, in_maps, core_ids=list(range(NCORES)), trace=_trace)
    last_perf = res

    outs = []
    for core in range(NCORES):
        ot = np.asarray(res.results[core]["outT"], dtype=np.float32)
        o = ot[:, :D, :] / ot[:, D:D + 1, :]           # normalize
        outs.append(o.transpose(0, 2, 1))              # [4, T, 64]
    full = np.concatenate(outs, axis=0)                # [32, T, 64]
    return full.reshape(B, H, T, D)


# revision 12
# speedup vs baseline: 2.6721x; 1.2646x over previous
"""LocalAttention Bass kernel for Trainium2 (8 NeuronCores).

Problem: B=4 H=8 T=8192 D=64, window=128, look_backward=1, causal.
Sharding: pure (B*H) data parallelism — 32 heads / 8 cores = 4 heads each,
processed as 2 head-pairs so q/k DMAs use all 128 SBUF partitions.

Device algorithm (per head, per 128-token window w):
  S^T[k, q] = K_w' @ Q_w^T      (keys on partitions, so the softmax
                                 reduction over keys can ride the PV matmul)
  P = exp(S^T * D^-0.5) * causal01
  [O^T; r] = [V | 1]^T @ P      (ones column baked into V gives row-sums)
Host divides O^T by r and transposes back.

v3 performance notes (fp32 baseline 305 us; v2 bf16 144 us):
  - all matmul operands bf16 (fp32 matmul = 4 cyc/col vs 1 for bf16)
  - PE warm-up burst at t=0: the HAM clock gate keeps an idle PE at
    1.2 GHz; ~7 us of dummy matmuls flips it to 2.4 GHz before the
    first real matmul (v2 ran 110 us of its 144 cold)
  - PV matmuls merged: the two windows sharing a V slot are computed by
    ONE N=256 matmul (pairblock P columns are contiguous); accumulation
    relies on PSUM has_written semantics (start=True on the group's
    first matmul clears the whole bank; later start=False matmuls
    overwrite untouched elements, accumulate touched ones) ->
    5 matmuls + 5 LDWEIGHTS per group-head instead of 8+8
  - software pipelining: group g+1's S matmuls are emitted BEFORE group
    g's PV so the in-order tensor queue never stalls the scalar engine;
    h0's sp is double-buffered (h1 hides behind h0's activation)
  - exp writes P directly as bf16; causal mask 1/3 vector + 2/3 gpsimd;
    output copy downcasts to bf16 (halves HBM writes)
  - CHUNK_W=16 with loads prefetched 2 groups early cuts the startup
    serial DMA from ~15 us to ~6 us

PSUM budget (8 banks): sp0 x2 bufs (4) + sp1 x1 (2) + op x2 (2).

Host-side shard prep (inside kernel(), numpy, all bf16):
  qTp [2, 128, 8320]  — head-pair Q^T (head A rows 0-63, head B rows
                        64-127), one zero window appended (lookahead pad)
  kT  [2, 128, 8192]  — head-pair K^T
  vp  [4, 128, 4225]  — per head: 65 V slots x [V(64) | 1], slot 0 zeroed
  mask01 [128, 128]   — within-window causal 0/1 (k <= q)
Output:
  outT [4, 65, 8192] bf16 — rows 0..63 unnormalized O^T, row 64 row-sums
"""

import numpy as np

B, H, T, D = 4, 8, 8192, 64
W = 128                     # window size
WIN = T // W                # 64 windows per head
NCORES = 8
BH = B * H                  # 32
BH_PER_CORE = BH // NCORES  # 4
NPAIR = BH_PER_CORE // 2    # 2 head pairs per core
CHUNK_W = 16                # windows per load chunk
NCHUNK = WIN // CHUNK_W     # 4
G = 4                       # windows per softmax group (PSUM tile = [128, 1024])
GPC = CHUNK_W // G          # groups per chunk
SCALE = float(D) ** -0.5

N_WARM = 16                 # PE warm-up matmuls (N=512 each)

_nc_cache = {}
last_perf = None


def _build_nc(skip=()):
    import concourse.tile as tile
    from concourse import bacc
    from concourse import mybir
    from contextlib import ExitStack

    f32 = mybir.dt.float32
    bf16 = mybir.dt.bfloat16
    Exp = mybir.ActivationFunctionType.Exp
    mult = mybir.AluOpType.mult

    nc = bacc.Bacc()
    qTp = nc.dram_tensor("qTp", [NPAIR, W, (WIN + 1) * W], bf16,
                         kind="ExternalInput")
    kT = nc.dram_tensor("kT", [NPAIR, W, T], bf16, kind="ExternalInput")
    vp = nc.dram_tensor("vp", [BH_PER_CORE, W, (WIN + 1) * (D + 1)], bf16,
                        kind="ExternalInput")
    mask = nc.dram_tensor("mask01", [W, W], bf16, kind="ExternalInput")
    outT = nc.dram_tensor("outT", [BH_PER_CORE, D + 1, T], bf16,
                          kind="ExternalOutput")

    with tile.TileContext(nc) as tc, ExitStack() as ctx:
        cpool = ctx.enter_context(tc.tile_pool(name="cpool", bufs=1))
        qpool = ctx.enter_context(tc.tile_pool(name="qpool", bufs=2))
        kpool = ctx.enter_context(tc.tile_pool(name="kpool", bufs=2))
        vpool = ctx.enter_context(tc.tile_pool(name="vpool", bufs=2))
        opool = ctx.enter_context(tc.tile_pool(name="opool", bufs=3))
        ppool = ctx.enter_context(tc.tile_pool(name="ppool", bufs=4))
        spsum0 = ctx.enter_context(tc.tile_pool(name="spsum0", bufs=2,
                                                space="PSUM"))
        spsum1 = ctx.enter_context(tc.tile_pool(name="spsum1", bufs=1,
                                                space="PSUM"))
        opsum = ctx.enter_context(tc.tile_pool(name="opsum", bufs=2,
                                               space="PSUM"))

        mtile = cpool.tile([W, W], bf16)
        nc.sync.dma_start(mtile[:], mask[:])
        z128 = cpool.tile([W, W], bf16)      # P for the all-masked pad window
        nc.vector.memset(z128[:], 0.0)

        mm = nc.tensor.matmul

        # --- PE warm-up: flip the HAM clock gate to 2.4 GHz while the ---
        # --- first input DMAs are in flight (results are discarded)   ---
        if "warm" not in skip and N_WARM:
            wz = cpool.tile([64, 4 * W], bf16)
            nc.vector.memset(wz[:], 0.0)
            wps = spsum0.tile([W, G * 2 * W], f32, tag="sp0")
            for _ in range(N_WARM):
                mm(wps[:, 0:4 * W], z128[0:64, :], wz[:],
                   start=True, stop=True)

        # flattened (chunk, group) schedule with S one group ahead of PV
        def s_phase(c, g, qc, kc, sps):
            """S^T pairblock matmuls for group (c, g): h0 block then h1."""
            w0 = g * G
            for h in range(2):
                hb = h * 64
                for i in range(G):
                    wl = w0 + i
                    mm(sps[h][:, i * 256:(i + 1) * 256],
                       kc[hb:hb + 64, wl * W:(wl + 1) * W],
                       qc[hb:hb + 64, wl * W:(wl + 2) * W],
                       start=True, stop=True)

        gidx = 0

        def consume_phase(p, c, g, vcs, ocs, sps, pt_prev):
            """exp + mask + merged PV + output copy for group (c, g)."""
            nonlocal gidx
            w0 = g * G
            for h in range(2):
                vc = vcs[h]
                pt = ppool.tile([W, G * 2 * W], bf16, tag=f"pt{h}",
                                name=f"pt{h}")
                if "exp" not in skip:
                    nc.scalar.activation(pt[:], sps[h][:], Exp, scale=SCALE)

                # causal mask on T1 blocks (cols 0,256,512,768)
                pt3 = pt[:].rearrange("p (g x) -> p g x", x=2 * W)
                t1 = pt3[:, :, 0:W]
                mb = mtile[:, None, :].to_broadcast([W, G, W])
                if "mask" not in skip:
                    if gidx % 3 == 0:
                        nc.vector.tensor_tensor(t1, t1, mb, mult)
                    else:
                        nc.gpsimd.tensor_tensor(t1, t1, mb, mult)

                # merged PV + row-sums: 5 matmuls instead of 8.
                # start=True on the boundary matmul clears the whole op
                # bank's has_written bits; the N=256 matmuls then overwrite
                # untouched columns and accumulate touched ones.
                op = opsum.tile([D + 1, G * W], f32, tag="op", name="op")
                if "pv" not in skip:
                    if pt_prev[h] is not None:
                        t0src = pt_prev[h][:, G * 256 - W:G * 256]
                    else:
                        t0src = z128[:]
                    mm(op[:, 0:W],
                       vc[:, w0 * (D + 1):(w0 + 1) * (D + 1)],
                       t0src, start=True, stop=False)
                    for j in range(1, G):
                        mm(op[:, (j - 1) * W:(j + 1) * W],
                           vc[:, (w0 + j) * (D + 1):(w0 + j + 1) * (D + 1)],
                           pt[:, (j - 1) * 256:j * 256],
                           start=False, stop=False)
                    mm(op[:, (G - 1) * W:G * W],
                       vc[:, (w0 + G) * (D + 1):(w0 + G + 1) * (D + 1)],
                       pt[:, (G - 1) * 256:(G - 1) * 256 + W],
                       start=False, stop=True)

                if "ocopy" not in skip:
                    nc.vector.tensor_copy(
                        ocs[h][:, w0 * W:(w0 + G) * W], op[:])
                pt_prev[h] = pt
                gidx += 1

        def load_chunk(p, c):
            c0 = c * CHUNK_W * W
            qc = qpool.tile([W, (CHUNK_W + 1) * W], bf16, tag="qc", name="qc")
            kc = kpool.tile([W, CHUNK_W * W], bf16, tag="kc", name="kc")
            if "loads" not in skip:
                nc.sync.dma_start(qc[:], qTp[p, :, c0:c0 + (CHUNK_W + 1) * W])
                nc.sync.dma_start(kc[:], kT[p, :, c0:c0 + CHUNK_W * W])
            vcs, ocs = [], []
            for h in range(2):
                vc = vpool.tile([W, (CHUNK_W + 1) * (D + 1)], bf16,
                                tag=f"vc{h}", name=f"vc{h}")
                if "loads" not in skip:
                    v0 = c * CHUNK_W * (D + 1)
                    nc.sync.dma_start(
                        vc[:],
                        vp[2 * p + h, :, v0:v0 + (CHUNK_W + 1) * (D + 1)])
                vcs.append(vc)
                oc = opool.tile([D + 1, CHUNK_W * W], bf16, tag="oc",
                                name="oc")
                ocs.append(oc)
            return dict(qc=qc, kc=kc, vcs=vcs, ocs=ocs, c=c, p=p)

        # global schedule
        sched = [(p, c, g) for p in range(NPAIR) for c in range(NCHUNK)
                 for g in range(GPC)]
        n = len(sched)
        chunks = {}          # (p, c) -> chunk tiles
        pt_prev_by_p = {p: [None, None] for p in range(NPAIR)}
        sp_of = {}           # i -> sps tiles for sched[i]

        def ensure_chunk(i):
            if i >= n:
                return
            p, c, g = sched[i]
            if (p, c) not in chunks:
                chunks[(p, c)] = load_chunk(p, c)

        ensure_chunk(0)

        def emit_s(i):
            p, c, g = sched[i]
            ck = chunks[(p, c)]
            sp0 = spsum0.tile([W, G * 2 * W], f32, tag="sp0", name="sp0")
            sp1 = spsum1.tile([W, G * 2 * W], f32, tag="sp1", name="sp1")
            sps = [sp0, sp1]
            sp_of[i] = sps
            if "smm" not in skip:
                s_phase(c, g, ck["qc"], ck["kc"], sps)

        emit_s(0)
        for i in range(n):
            p, c, g = sched[i]
            # prefetch the chunk needed 2 groups ahead
            ensure_chunk(i + 2)
            # S matmuls for the NEXT group go in front of this group's PV
            if i + 1 < n:
                emit_s(i + 1)
            ck = chunks[(p, c)]
            pt_prev = pt_prev_by_p[p]
            consume_phase(p, c, g, ck["vcs"], ck["ocs"], sp_of.pop(i),
                          pt_prev)
            # chunk finished? store its outputs; reset pt_prev across pairs
            if g == GPC - 1:
                if "store" not in skip:
                    c0 = c * CHUNK_W * W
                    for h in range(2):
                        nc.sync.dma_start(
                            outT[2 * p + h, :, c0:c0 + CHUNK_W * W],
                            ck["ocs"][h][:])
                if c == NCHUNK - 1:
                    pt_prev_by_p[p] = [None, None]
    nc.finalize()
    return nc


def _prep_core_inputs(q2, k2, v2, core):
    import ml_dtypes
    bf16 = ml_dtypes.bfloat16
    s0 = core * BH_PER_CORE
    qTp = np.zeros((NPAIR, W, (WIN + 1) * W), bf16)
    kTp = np.zeros((NPAIR, W, T), bf16)
    for p in range(NPAIR):
        for h in range(2):
            bh = s0 + 2 * p + h
            qTp[p, h * 64:(h + 1) * 64, :T] = q2[bh].T.astype(bf16)
            kTp[p, h * 64:(h + 1) * 64, :] = k2[bh].T.astype(bf16)
    vr = v2[s0:s0 + BH_PER_CORE].reshape(
        BH_PER_CORE, WIN, W, D).transpose(0, 2, 1, 3)
    vp = np.zeros((BH_PER_CORE, W, WIN + 1, D + 1), bf16)
    vp[:, :, 1:, :D] = vr.astype(bf16)
    vp[:, :, :, D] = 1.0
    vp = np.ascontiguousarray(vp.reshape(BH_PER_CORE, W, (WIN + 1) * (D + 1)))
    mask01 = (np.arange(W)[:, None] <= np.arange(W)[None, :]).astype(bf16)
    return {"qTp": qTp, "kT": kTp, "vp": vp, "mask01": mask01}


def kernel(q, k, v, _trace=False):
    global last_perf
    from concourse.bass_utils import run_bass_kernel_spmd

    q = np.ascontiguousarray(np.asarray(q), dtype=np.float32)
    k = np.ascontiguousarray(np.asarray(k), dtype=np.float32)
    v = np.ascontiguousarray(np.asarray(v), dtype=np.float32)
    q2 = q.reshape(BH, T, D)
    k2 = k.reshape(BH, T, D)
    v2 = v.reshape(BH, T, D)

    if "nc" not in _nc_cache:
        _nc_cache["nc"] = _build_nc()
    nc = _nc_cache["nc"]

    in_maps = [_prep_core_inputs(q2, k2, v2, core) for core in range(NCORES)]
    res = run_bass_kernel_spmd(
        nc, in_maps, core_ids=list(range(NCORES)), trace=_trace)
    last_perf = res

    outs = []
    for core in range(NCORES):
        ot = np.asarray(res.results[core]["outT"], dtype=np.float32)
        o = ot[:, :D, :] / ot[:, D:D + 1, :]           # normalize
        outs.append(o.transpose(0, 2, 1))              # [4, T, 64]
    full = np.concatenate(outs, axis=0)                # [32, T, 64]
    return full.reshape(B, H, T, D)
